# revision 27
# baseline (speedup 1.0000x reference)
"""Trainium2 Bass kernel for nn_ContrastiveMSELoss (8192x8192 cos-sim contrastive + MSE).

Sharding: 8 NeuronCores, users row-sharded 1024/core, full (normalized,
transposed) recipe table per core.

The loss decomposes so the 8192x8192 ratings matrix is never materialized:
    rowR[i]  = 0.1*M + sum_{final scatter cells in row i}(v - 0.1)
    S1       = 0.1*T + sum_pairs (v-0.1)*cos[u,i],  T = (sum_i u_i/|u_i|) . (sum_j r_j/|r_j|)
    S2       = sum_i rowR[i] * log(rowsum_exp[i])
    S3       = sum_i rowR[i] * log(colsum_exp[i])    (col_sum indexed by i: torch n==m quirk)
    loss     = 0.5*(S2 + S3 - 2*S1)/(2*N) + 0.5*mean((ratings-cos_sim)^2)

v9: host prep in numpy (normalize+transpose embeddings, dedup scatter, and the
O(N*D)/O(B*D) scalar terms PAIR/T/MSE in f64); the device keeps the O(N^2) work.
Main loop, 4 column groups x 8 row tiles: K=64 bf16 matmuls into a double-
buffered [128,2048] f32 PSUM ring, one wide Exp ACTIVATE per tile (ACT is the
throughput floor: 8.4M exps/core at 1 elem/cycle/lane @1.2GHz) with row sums on
the ACT accumulator, column sums accumulated on DVE in bf16. Each group's
partition-reduce (col-tiled ones-matmuls into a pg-ring slot + bounce + DMA) is
deferred to the middle of the NEXT group, emitted before that iteration's pg so
it lands in an already-free ring slot. Cross-core column reduction: warmup
AllGather at t~0 (absorbs the CC cold barrier), stage-A AllGather mid-loop
(hidden), stage-B exposed at the tail with the last ex tile folded into the
reduce on PE, the PSUM bounce split across ACT+DVE, cc_in DMAs split across both
HWDGE queues, and every core redundantly reducing all 8 partials (no
core-dependent addressing). S2/S3 partials exit via one ones-matmul + DMA.
"""

import sys

sys.path.insert(0, "/opt/trn_rl_repo")

import numpy as np

import concourse.bass as bass
import concourse.bacc as bacc
import concourse.tile as tile
from concourse import mybir
from concourse.bass_utils import run_bass_kernel_spmd

f32 = mybir.dt.float32
bf16 = mybir.dt.bfloat16
AF = mybir.ActivationFunctionType
OP = mybir.AluOpType
AX = mybir.AxisListType

NCORES = 8
N = 8192          # users
M = 8192          # recipes
D = 64
B = 65536
S = N // NCORES   # slab rows per core (1024)
RT = S // 128     # row tiles per slab (8)
NG = 4            # column groups
GW = 2048         # group width (cols)
ALPHA = 0.5
FILL = 0.1

CS_P = (0, 32, 64, 0)    # ones-matmul output partition per 512-chunk
CS_F = (0, 0, 0, 512)    # ... and free offset (base_partition 96 is rejected)


def build_nc():
    nc = bacc.Bacc(num_devices=NCORES)

    ut_d = nc.declare_dram_parameter("ut", [D, S], bf16, isOutput=False)
    rt_d = nc.declare_dram_parameter("rt", [D, M], bf16, isOutput=False)
    rowr_slab_d = nc.declare_dram_parameter("rowr_slab", [S], f32, isOutput=False)
    rowr_full_d = nc.declare_dram_parameter("rowr_full", [M], f32, isOutput=False)
    out_d = nc.declare_dram_parameter("out", [128, 3], f32, isOutput=True)

    with tile.TileContext(nc) as tc:
        with tc.tile_pool(name="sb", bufs=1) as sb, \
             tc.tile_pool(name="dram", bufs=1, space="DRAM") as dpool:
            cc_in = dpool.tile([M], f32)
            cc_allA = dpool.tile([NCORES, 4096], f32)
            cc_allB = dpool.tile([NCORES, 4096], f32)
            warm_in = dpool.tile([16], f32)
            warm_out = dpool.tile([16 * NCORES], f32)

            # ---- highest-priority loads: exactly what the first matmuls
            # need first, split across both HWDGE queues ----
            ut_sb = sb.tile([D, S], bf16)
            rt_sb = sb.tile([D, M], bf16)
            nc.sync.dma_start(out=ut_sb[:, 0:128], in_=ut_d[:, 0:128])
            nc.scalar.dma_start(out=rt_sb[:, 1024:2048], in_=rt_d[:, 1024:2048])
            nc.sync.dma_start(out=rt_sb[:, 0:512], in_=rt_d[:, 0:512])
            nc.sync.dma_start(out=rt_sb[:, 512:1024], in_=rt_d[:, 512:1024])
            nc.sync.dma_start(out=ut_sb[:, 128:S], in_=ut_d[:, 128:S])

            ones_bf = sb.tile([128, 1], bf16)
            nc.vector.memset(ones_bf[:], 1.0)

            # prime the CC channel early; nothing depends on the result
            wz = sb.tile([1, 16], f32)
            nc.vector.memset(wz[:], 0.0)
            nc.sync.dma_start(out=warm_in[:], in_=wz[:])
            nc.gpsimd.collective_compute(
                "AllGather", OP.bypass,
                replica_groups=[list(range(NCORES))],
                ins=[warm_in[:].opt()], outs=[warm_out[:].opt()])

            for g in range(1, NG):
                nc.sync.dma_start(
                    out=rt_sb[:, g * GW:(g + 1) * GW], in_=rt_d[:, g * GW:(g + 1) * GW])
            rowr2 = sb.tile([128, RT], f32)
            nc.sync.dma_start(out=rowr2[:], in_=rowr_slab_d[:].rearrange("(r p) -> p r", p=128))
            # rowR keyed by colsum index, per AllGather stage (A: cols 0-4096,
            # B: 4096-8192); within a stage col j = off + p*32 + b
            rowrA = sb.tile([128, 32], f32)
            nc.sync.dma_start(out=rowrA[:], in_=rowr_full_d[0:4096].rearrange("(p b) -> p b", p=128))
            rowrB = sb.tile([128, 32], f32)
            nc.sync.dma_start(out=rowrB[:], in_=rowr_full_d[4096:8192].rearrange("(p b) -> p b", p=128))

            rs_parts = sb.tile([128, NG * RT], f32)
            acc = sb.tile([128, NG, GW], bf16)
            s3_acc = sb.tile([128, 2], f32)

            with tc.tile_pool(name="psM", bufs=1, space="PSUM") as psM:

                def colsum_reduce(g, tail=False, extra=None):
                    """Partition-reduce acc[:, g] (128 -> 1) via 4 col-tiled
                    ones-matmuls into a pg-ring PSUM slot, bounce, DMA to cc_in.
                    `extra` is an unaccumulated ex tile folded in via a second
                    accumulating matmul (keeps the last DVE add off the tail)."""
                    csP = psM.tile([128, GW], f32, tag="cos", bufs=2, name=f"cs{g}")
                    for c in range(4):
                        nc.tensor.matmul(
                            out=csP[CS_P[c]:CS_P[c] + 1, CS_F[c]:CS_F[c] + 512],
                            lhsT=ones_bf[:, 0:1],
                            rhs=acc[:, g, c * 512:(c + 1) * 512],
                            start=True, stop=extra is None, skip_group_check=True)
                        if extra is not None:
                            nc.tensor.matmul(
                                out=csP[CS_P[c]:CS_P[c] + 1, CS_F[c]:CS_F[c] + 512],
                                lhsT=ones_bf[:, 0:1],
                                rhs=extra[:, c * 512:(c + 1) * 512],
                                start=False, stop=True, skip_group_check=True)
                    # split the PSUM->SBUF bounce across ACT+DVE: mid-loop ACT
                    # is stalled on the ring handoff here anyway (Copy is in
                    # the Exp table set, no reload), and the earlier bounce
                    # completion releases the ring slot sooner
                    bounce = sb.tile([128, 1024], f32, tag="csb", bufs=2, name=f"csb{g}")
                    nc.scalar.activation(
                        out=bounce[:, 0:512], in_=csP[:, 0:512], func=AF.Copy)
                    nc.vector.tensor_copy(out=bounce[:, 512:1024], in_=csP[:, 512:1024])
                    for c in range(4):
                        eng = nc.scalar if (tail and c % 2) else nc.sync
                        eng.dma_start(
                            out=cc_in[g * GW + c * 512: g * GW + (c + 1) * 512],
                            in_=bounce[CS_P[c]:CS_P[c] + 1, CS_F[c]:CS_F[c] + 512])

                for g in range(NG):
                    for r in range(RT):
                        # deferred reduce of the PREVIOUS group, emitted before
                        # this iteration's pg so csP lands in the ring slot
                        # whose ACT reader finished two iterations ago
                        if r == 4 and g >= 1:
                            colsum_reduce(g - 1)
                            if g == 2:
                                nc.gpsimd.collective_compute(
                                    "AllGather", OP.bypass,
                                    replica_groups=[list(range(NCORES))],
                                    ins=[cc_in[0:4096].opt()], outs=[cc_allA[:].opt()])
                        pg = psM.tile([128, GW], f32, tag="cos", bufs=2)
                        for c in range(4):
                            nc.tensor.matmul(
                                out=pg[:, c * 512:(c + 1) * 512],
                                lhsT=ut_sb[:, r * 128:(r + 1) * 128],
                                rhs=rt_sb[:, g * GW + c * 512: g * GW + (c + 1) * 512],
                                start=True, stop=True)
                        ex = sb.tile([128, GW], bf16, tag="exp", bufs=4)
                        idx = g * RT + r
                        nc.scalar.activation(
                            out=ex[:], in_=pg[:], func=AF.Exp,
                            accum_out=rs_parts[:, idx:idx + 1])
                        if r == 0:
                            nc.vector.tensor_copy(out=acc[:, g, :], in_=ex[:])
                        elif g == NG - 1 and r == RT - 1:
                            last_ex = ex  # folded into colsum_reduce(3) on PE
                        else:
                            nc.vector.tensor_tensor(
                                out=acc[:, g, :], in0=acc[:, g, :], in1=ex[:], op=OP.add)

                colsum_reduce(3, tail=True, extra=last_ex)
                nc.gpsimd.collective_compute(
                    "AllGather", OP.bypass,
                    replica_groups=[list(range(NCORES))],
                    ins=[cc_in[4096:8192].opt()], outs=[cc_allB[:].opt()])

            # =============== tail ===============
            if True:
                # stage-A gather finished mid-loop; its local reduce + ln +
                # dot run here, under the AG-B latency. (Kept out of the main
                # loop: a slow collective there blocks the DVE queue and
                # backs up into ACT.)
                tmpA = sb.tile([128, NCORES, 32], f32)
                nc.sync.dma_start(
                    out=tmpA[:], in_=cc_allA[:].rearrange("c (p b) -> p c b", p=128))
                colsA = sb.tile([128, 32], f32)
                nc.vector.tensor_reduce(
                    out=colsA[:], in_=tmpA[:].rearrange("p c b -> p b c"),
                    axis=AX.X, op=OP.add)

                # S2 path: runs on ACT/DVE while AllGather C is in flight
                rs_r = sb.tile([128, RT], f32)
                nc.vector.tensor_reduce(
                    out=rs_r[:], in_=rs_parts[:].rearrange("p (g r) -> p r g", g=NG),
                    axis=AX.X, op=OP.add)
                lrs = sb.tile([128, RT], f32)
                nc.scalar.activation(out=lrs[:], in_=rs_r[:], func=AF.Ln)
                s2w = sb.tile([128, RT], f32)
                nc.vector.tensor_tensor(out=s2w[:], in0=lrs[:], in1=rowr2[:], op=OP.mult)
                s2_acc = sb.tile([128, 1], f32)
                nc.vector.tensor_reduce(out=s2_acc[:], in_=s2w[:], axis=AX.X, op=OP.add)

                lnA = sb.tile([128, 32], f32)
                nc.scalar.activation(out=lnA[:], in_=colsA[:], func=AF.Ln)
                w3A = sb.tile([128, 32], f32)
                nc.vector.tensor_tensor(out=w3A[:], in0=lnA[:], in1=rowrA[:], op=OP.mult)
                nc.vector.tensor_reduce(out=s3_acc[:, 0:1], in_=w3A[:], axis=AX.X, op=OP.add)
                # stage B: gather + reduce + ln + dot (exposed)
                tmpB = sb.tile([128, NCORES, 32], f32)
                nc.sync.dma_start(
                    out=tmpB[:], in_=cc_allB[:].rearrange("c (p b) -> p c b", p=128))
                colsB = sb.tile([128, 32], f32)
                nc.vector.tensor_reduce(
                    out=colsB[:], in_=tmpB[:].rearrange("p c b -> p b c"),
                    axis=AX.X, op=OP.add)
                lnB = sb.tile([128, 32], f32)
                nc.scalar.activation(out=lnB[:], in_=colsB[:], func=AF.Ln)
                w3B = sb.tile([128, 32], f32)
                nc.vector.tensor_tensor(out=w3B[:], in0=lnB[:], in1=rowrB[:], op=OP.mult)
                nc.vector.tensor_reduce(out=s3_acc[:, 1:2], in_=w3B[:], axis=AX.X, op=OP.add)

                # ship [128,3] per-partition partials; host sums them
                combo = sb.tile([128, 3], f32)
                nc.vector.tensor_copy(out=combo[:, 0:1], in_=s2_acc[:])
                nc.vector.tensor_copy(out=combo[:, 1:3], in_=s3_acc[:])
                nc.sync.dma_start(out=out_d[:], in_=combo[:])
    nc.finalize()
    return nc


def _host_prep(inputs):
    """Normalize+transpose embeddings, dedup scatter (last write wins), and the
    O(N*D)/O(B*D) scalar terms (PAIR, T, MSE) in f64 numpy."""
    U = np.asarray(inputs["user_embeddings"], dtype=np.float32)
    R = np.asarray(inputs["recipe_embeddings"], dtype=np.float32)
    rat = np.asarray(inputs["ratings_scaled"], dtype=np.float32)
    css = np.asarray(inputs["cos_similarities_scaled"], dtype=np.float32)
    u = np.asarray(inputs["u_idx"]).astype(np.int64)
    i = np.asarray(inputs["i_idx"]).astype(np.int64)

    Uh = U.astype(np.float64)
    Rh = R.astype(np.float64)
    un = np.maximum(np.linalg.norm(Uh, axis=1), 1e-8)
    rn = np.maximum(np.linalg.norm(Rh, axis=1), 1e-8)
    Uh /= un[:, None]
    Rh /= rn[:, None]

    import ml_dtypes
    ut_all = np.ascontiguousarray(Uh.T).astype(ml_dtypes.bfloat16)   # [64, N]
    rt = np.ascontiguousarray(Rh.T).astype(ml_dtypes.bfloat16)       # [64, M]

    cell = u * M + i
    _, idx_rev = np.unique(cell[::-1], return_index=True)
    keep = (B - 1 - idx_rev)  # last occurrences
    uu = u[keep]
    ii = i[keep]
    ww = (rat[keep].astype(np.float64) - FILL)

    delta = np.bincount(uu, weights=ww, minlength=N)
    row_r32 = (FILL * M + delta).astype(np.float32)

    pair = float(np.einsum("ij,ij->", Uh[uu] * ww[:, None], Rh[ii]))
    T = float(Uh.sum(0) @ Rh.sum(0))
    s1 = FILL * T + pair
    mse = float(np.mean((rat.astype(np.float64) - css.astype(np.float64)) ** 2))

    in_maps = []
    for c in range(NCORES):
        in_maps.append({
            "ut": np.ascontiguousarray(ut_all[:, c * S:(c + 1) * S]),
            "rt": rt,
            "rowr_slab": np.ascontiguousarray(row_r32[c * S:(c + 1) * S]),
            "rowr_full": row_r32,
        })
    return in_maps, s1, mse


def kernel(user_embeddings, recipe_embeddings, ratings_scaled, cos_similarities_scaled,
           u_idx, i_idx, _trace=False):
    inputs = {
        "user_embeddings": user_embeddings,
        "recipe_embeddings": recipe_embeddings,
        "ratings_scaled": ratings_scaled,
        "cos_similarities_scaled": cos_similarities_scaled,
        "u_idx": u_idx,
        "i_idx": i_idx,
    }
    in_maps, s1, mse = _host_prep(inputs)
    nc = build_nc()
    res = run_bass_kernel_spmd(nc, in_maps, core_ids=list(range(NCORES)), trace=_trace)
    outs = np.stack([res.results[c]["out"] for c in range(NCORES)]).astype(np.float64)  # [8,128,3]
    S2 = outs[..., 0].sum()
    S3 = outs[..., 1:3].sum() / NCORES  # redundant per-core copies
    contrastive = (S2 + S3 - 2.0 * s1) / (2.0 * N)
    loss = ALPHA * contrastive + (1.0 - ALPHA) * mse
    if _trace:
        kernel._last_results = res
    return np.float32(loss)


# revision 28
# speedup vs baseline: 1.8261x; 1.8261x over previous
"""Trainium2 Bass kernel for nn_ContrastiveMSELoss (8192x8192 cos-sim contrastive + MSE).

Sharding: 8 NeuronCores, users row-sharded 1024/core, full (normalized,
transposed) recipe table per core.

The loss decomposes so the 8192x8192 ratings matrix is never materialized:
    rowR[i]  = 0.1*M + sum_{final scatter cells in row i}(v - 0.1)
    S1       = 0.1*T + sum_pairs (v-0.1)*cos[u,i],  T = (sum_i u_i/|u_i|) . (sum_j r_j/|r_j|)
    S2       = sum_i rowR[i] * log(rowsum_exp[i])
    S3       = sum_i rowR[i] * log(colsum_exp[i])    (col_sum indexed by i: torch n==m quirk)
    loss     = 0.5*(S2 + S3 - 2*S1)/(2*N) + 0.5*mean((ratings-cos_sim)^2)

v9: host prep in numpy (normalize+transpose embeddings, dedup scatter, and the
O(N*D)/O(B*D) scalar terms PAIR/T/MSE in f64); the device keeps the O(N^2) work.
Main loop, 4 column groups x 8 row tiles: K=64 bf16 matmuls into a double-
buffered [128,2048] f32 PSUM ring, one wide Exp ACTIVATE per tile (ACT is the
throughput floor: 8.4M exps/core at 1 elem/cycle/lane @1.2GHz) with row sums on
the ACT accumulator, column sums accumulated on DVE in bf16. Each group's
partition-reduce (col-tiled ones-matmuls into a pg-ring slot + bounce + DMA) is
deferred to the middle of the NEXT group, emitted before that iteration's pg so
it lands in an already-free ring slot. Cross-core column reduction: warmup
AllGather at t~0 (absorbs the CC cold barrier), stage-A AllGather mid-loop
(hidden), stage-B exposed at the tail with the last ex tile folded into the
reduce on PE, the PSUM bounce split across ACT+DVE, cc_in DMAs split across both
HWDGE queues, and every core redundantly reducing all 8 partials (no
core-dependent addressing). S2/S3 exit as [128,3] per-partition partials in one
DMA; the host does the final partition/core sums in f64.
"""

import sys

sys.path.insert(0, "/opt/trn_rl_repo")

import numpy as np

import concourse.bass as bass
import concourse.bacc as bacc
import concourse.tile as tile
from concourse import mybir
from concourse.bass_utils import run_bass_kernel_spmd

f32 = mybir.dt.float32
bf16 = mybir.dt.bfloat16
AF = mybir.ActivationFunctionType
OP = mybir.AluOpType
AX = mybir.AxisListType

NCORES = 8
N = 8192          # users
M = 8192          # recipes
D = 64
B = 65536
S = N // NCORES   # slab rows per core (1024)
RT = S // 128     # row tiles per slab (8)
NG = 4            # column groups
GW = 2048         # group width (cols)
ALPHA = 0.5
FILL = 0.1

CS_P = (0, 32, 64, 0)    # ones-matmul output partition per 512-chunk
CS_F = (0, 0, 0, 512)    # ... and free offset (base_partition 96 is rejected)


def build_nc():
    nc = bacc.Bacc(num_devices=NCORES)

    ut_d = nc.declare_dram_parameter("ut", [D, S], bf16, isOutput=False)
    rt_d = nc.declare_dram_parameter("rt", [D, M], bf16, isOutput=False)
    rowr_slab_d = nc.declare_dram_parameter("rowr_slab", [S], f32, isOutput=False)
    rowr_full_d = nc.declare_dram_parameter("rowr_full", [M], f32, isOutput=False)
    out_d = nc.declare_dram_parameter("out", [128, 3], f32, isOutput=True)

    with tile.TileContext(nc) as tc:
        with tc.tile_pool(name="sb", bufs=1) as sb, \
             tc.tile_pool(name="dram", bufs=1, space="DRAM") as dpool:
            cc_in = dpool.tile([M], f32)
            cc_allA = dpool.tile([NCORES, 4096], f32)
            cc_allB = dpool.tile([NCORES, 4096], f32)
            warm_in = dpool.tile([16], f32)
            warm_out = dpool.tile([16 * NCORES], f32)

            # ---- highest-priority loads: exactly what the first matmuls
            # need first, split across both HWDGE queues ----
            ut_sb = sb.tile([D, S], bf16)
            rt_sb = sb.tile([D, M], bf16)
            nc.sync.dma_start(out=ut_sb[:, 0:128], in_=ut_d[:, 0:128])
            nc.scalar.dma_start(out=rt_sb[:, 1024:2048], in_=rt_d[:, 1024:2048])
            nc.sync.dma_start(out=rt_sb[:, 0:512], in_=rt_d[:, 0:512])
            nc.sync.dma_start(out=rt_sb[:, 512:1024], in_=rt_d[:, 512:1024])
            nc.sync.dma_start(out=ut_sb[:, 128:S], in_=ut_d[:, 128:S])

            ones_bf = sb.tile([128, 1], bf16)
            nc.vector.memset(ones_bf[:], 1.0)

            # prime the CC channel early; nothing depends on the result
            wz = sb.tile([1, 16], f32)
            nc.vector.memset(wz[:], 0.0)
            nc.sync.dma_start(out=warm_in[:], in_=wz[:])
            nc.gpsimd.collective_compute(
                "AllGather", OP.bypass,
                replica_groups=[list(range(NCORES))],
                ins=[warm_in[:].opt()], outs=[warm_out[:].opt()])

            for g in range(1, NG):
                nc.sync.dma_start(
                    out=rt_sb[:, g * GW:(g + 1) * GW], in_=rt_d[:, g * GW:(g + 1) * GW])
            rowr2 = sb.tile([128, RT], f32)
            nc.sync.dma_start(out=rowr2[:], in_=rowr_slab_d[:].rearrange("(r p) -> p r", p=128))
            # rowR keyed by colsum index, per AllGather stage (A: cols 0-4096,
            # B: 4096-8192); within a stage col j = off + p*32 + b
            rowrA = sb.tile([128, 32], f32)
            nc.sync.dma_start(out=rowrA[:], in_=rowr_full_d[0:4096].rearrange("(p b) -> p b", p=128))
            rowrB = sb.tile([128, 32], f32)
            nc.sync.dma_start(out=rowrB[:], in_=rowr_full_d[4096:8192].rearrange("(p b) -> p b", p=128))

            rs_parts = sb.tile([128, NG * RT], f32)
            acc = sb.tile([128, NG, GW], bf16)
            s3_acc = sb.tile([128, 2], f32)

            with tc.tile_pool(name="psM", bufs=1, space="PSUM") as psM:

                def colsum_reduce(g, tail=False, extra=None):
                    """Partition-reduce acc[:, g] (128 -> 1) via 4 col-tiled
                    ones-matmuls into a pg-ring PSUM slot, bounce, DMA to cc_in.
                    `extra` is an unaccumulated ex tile folded in via a second
                    accumulating matmul (keeps the last DVE add off the tail)."""
                    csP = psM.tile([128, GW], f32, tag="cos", bufs=2, name=f"cs{g}")
                    for c in range(4):
                        nc.tensor.matmul(
                            out=csP[CS_P[c]:CS_P[c] + 1, CS_F[c]:CS_F[c] + 512],
                            lhsT=ones_bf[:, 0:1],
                            rhs=acc[:, g, c * 512:(c + 1) * 512],
                            start=True, stop=extra is None, skip_group_check=True)
                        if extra is not None:
                            nc.tensor.matmul(
                                out=csP[CS_P[c]:CS_P[c] + 1, CS_F[c]:CS_F[c] + 512],
                                lhsT=ones_bf[:, 0:1],
                                rhs=extra[:, c * 512:(c + 1) * 512],
                                start=False, stop=True, skip_group_check=True)
                    # split the PSUM->SBUF bounce across ACT+DVE: mid-loop ACT
                    # is stalled on the ring handoff here anyway (Copy is in
                    # the Exp table set, no reload), and the earlier bounce
                    # completion releases the ring slot sooner
                    bounce = sb.tile([128, 1024], f32, tag="csb", bufs=2, name=f"csb{g}")
                    nc.scalar.activation(
                        out=bounce[:, 0:512], in_=csP[:, 0:512], func=AF.Copy)
                    nc.vector.tensor_copy(out=bounce[:, 512:1024], in_=csP[:, 512:1024])
                    for c in range(4):
                        eng = nc.scalar if (tail and c % 2) else nc.sync
                        eng.dma_start(
                            out=cc_in[g * GW + c * 512: g * GW + (c + 1) * 512],
                            in_=bounce[CS_P[c]:CS_P[c] + 1, CS_F[c]:CS_F[c] + 512])

                for g in range(NG):
                    for r in range(RT):
                        # deferred reduce of the PREVIOUS group, emitted before
                        # this iteration's pg so csP lands in the ring slot
                        # whose ACT reader finished two iterations ago
                        if r == 4 and g >= 1:
                            colsum_reduce(g - 1)
                            if g == 2:
                                nc.gpsimd.collective_compute(
                                    "AllGather", OP.bypass,
                                    replica_groups=[list(range(NCORES))],
                                    ins=[cc_in[0:4096].opt()], outs=[cc_allA[:].opt()])
                        pg = psM.tile([128, GW], f32, tag="cos", bufs=2)
                        for c in range(4):
                            nc.tensor.matmul(
                                out=pg[:, c * 512:(c + 1) * 512],
                                lhsT=ut_sb[:, r * 128:(r + 1) * 128],
                                rhs=rt_sb[:, g * GW + c * 512: g * GW + (c + 1) * 512],
                                start=True, stop=True)
                        ex = sb.tile([128, GW], bf16, tag="exp", bufs=4)
                        idx = g * RT + r
                        nc.scalar.activation(
                            out=ex[:], in_=pg[:], func=AF.Exp,
                            accum_out=rs_parts[:, idx:idx + 1])
                        if r == 0:
                            nc.vector.tensor_copy(out=acc[:, g, :], in_=ex[:])
                        elif g == NG - 1 and r == RT - 1:
                            last_ex = ex  # folded into colsum_reduce(3) on PE
                        else:
                            nc.vector.tensor_tensor(
                                out=acc[:, g, :], in0=acc[:, g, :], in1=ex[:], op=OP.add)

                colsum_reduce(3, tail=True, extra=last_ex)
                nc.gpsimd.collective_compute(
                    "AllGather", OP.bypass,
                    replica_groups=[list(range(NCORES))],
                    ins=[cc_in[4096:8192].opt()], outs=[cc_allB[:].opt()])

            # =============== tail ===============
            if True:
                # stage-A gather finished mid-loop; its local reduce + ln +
                # dot run here, under the AG-B latency. (Kept out of the main
                # loop: a slow collective there blocks the DVE queue and
                # backs up into ACT.)
                tmpA = sb.tile([128, NCORES, 32], f32)
                nc.sync.dma_start(
                    out=tmpA[:], in_=cc_allA[:].rearrange("c (p b) -> p c b", p=128))
                colsA = sb.tile([128, 32], f32)
                nc.vector.tensor_reduce(
                    out=colsA[:], in_=tmpA[:].rearrange("p c b -> p b c"),
                    axis=AX.X, op=OP.add)

                # S2 path: runs on ACT/DVE while AllGather C is in flight
                rs_r = sb.tile([128, RT], f32)
                nc.vector.tensor_reduce(
                    out=rs_r[:], in_=rs_parts[:].rearrange("p (g r) -> p r g", g=NG),
                    axis=AX.X, op=OP.add)
                lrs = sb.tile([128, RT], f32)
                nc.scalar.activation(out=lrs[:], in_=rs_r[:], func=AF.Ln)
                s2w = sb.tile([128, RT], f32)
                nc.vector.tensor_tensor(out=s2w[:], in0=lrs[:], in1=rowr2[:], op=OP.mult)
                s2_acc = sb.tile([128, 1], f32)
                nc.vector.tensor_reduce(out=s2_acc[:], in_=s2w[:], axis=AX.X, op=OP.add)

                lnA = sb.tile([128, 32], f32)
                nc.scalar.activation(out=lnA[:], in_=colsA[:], func=AF.Ln)
                w3A = sb.tile([128, 32], f32)
                nc.vector.tensor_tensor(out=w3A[:], in0=lnA[:], in1=rowrA[:], op=OP.mult)
                nc.vector.tensor_reduce(out=s3_acc[:, 0:1], in_=w3A[:], axis=AX.X, op=OP.add)
                # stage B: gather + reduce + ln + dot (exposed)
                tmpB = sb.tile([128, NCORES, 32], f32)
                nc.sync.dma_start(
                    out=tmpB[:], in_=cc_allB[:].rearrange("c (p b) -> p c b", p=128))
                colsB = sb.tile([128, 32], f32)
                nc.vector.tensor_reduce(
                    out=colsB[:], in_=tmpB[:].rearrange("p c b -> p b c"),
                    axis=AX.X, op=OP.add)
                lnB = sb.tile([128, 32], f32)
                nc.scalar.activation(out=lnB[:], in_=colsB[:], func=AF.Ln)
                w3B = sb.tile([128, 32], f32)
                nc.vector.tensor_tensor(out=w3B[:], in0=lnB[:], in1=rowrB[:], op=OP.mult)
                nc.vector.tensor_reduce(out=s3_acc[:, 1:2], in_=w3B[:], axis=AX.X, op=OP.add)

                # ship [128,3] per-partition partials; host sums them
                combo = sb.tile([128, 3], f32)
                nc.vector.tensor_copy(out=combo[:, 0:1], in_=s2_acc[:])
                nc.vector.tensor_copy(out=combo[:, 1:3], in_=s3_acc[:])
                nc.sync.dma_start(out=out_d[:], in_=combo[:])
    nc.finalize()
    return nc


def _host_prep(inputs):
    """Normalize+transpose embeddings, dedup scatter (last write wins), and the
    O(N*D)/O(B*D) scalar terms (PAIR, T, MSE) in f64 numpy."""
    U = np.asarray(inputs["user_embeddings"], dtype=np.float32)
    R = np.asarray(inputs["recipe_embeddings"], dtype=np.float32)
    rat = np.asarray(inputs["ratings_scaled"], dtype=np.float32)
    css = np.asarray(inputs["cos_similarities_scaled"], dtype=np.float32)
    u = np.asarray(inputs["u_idx"]).astype(np.int64)
    i = np.asarray(inputs["i_idx"]).astype(np.int64)

    Uh = U.astype(np.float64)
    Rh = R.astype(np.float64)
    un = np.maximum(np.linalg.norm(Uh, axis=1), 1e-8)
    rn = np.maximum(np.linalg.norm(Rh, axis=1), 1e-8)
    Uh /= un[:, None]
    Rh /= rn[:, None]

    import ml_dtypes
    ut_all = np.ascontiguousarray(Uh.T).astype(ml_dtypes.bfloat16)   # [64, N]
    rt = np.ascontiguousarray(Rh.T).astype(ml_dtypes.bfloat16)       # [64, M]

    cell = u * M + i
    _, idx_rev = np.unique(cell[::-1], return_index=True)
    keep = (B - 1 - idx_rev)  # last occurrences
    uu = u[keep]
    ii = i[keep]
    ww = (rat[keep].astype(np.float64) - FILL)

    delta = np.bincount(uu, weights=ww, minlength=N)
    row_r32 = (FILL * M + delta).astype(np.float32)

    pair = float(np.einsum("ij,ij->", Uh[uu] * ww[:, None], Rh[ii]))
    T = float(Uh.sum(0) @ Rh.sum(0))
    s1 = FILL * T + pair
    mse = float(np.mean((rat.astype(np.float64) - css.astype(np.float64)) ** 2))

    in_maps = []
    for c in range(NCORES):
        in_maps.append({
            "ut": np.ascontiguousarray(ut_all[:, c * S:(c + 1) * S]),
            "rt": rt,
            "rowr_slab": np.ascontiguousarray(row_r32[c * S:(c + 1) * S]),
            "rowr_full": row_r32,
        })
    return in_maps, s1, mse


def kernel(user_embeddings, recipe_embeddings, ratings_scaled, cos_similarities_scaled,
           u_idx, i_idx, _trace=False):
    inputs = {
        "user_embeddings": user_embeddings,
        "recipe_embeddings": recipe_embeddings,
        "ratings_scaled": ratings_scaled,
        "cos_similarities_scaled": cos_similarities_scaled,
        "u_idx": u_idx,
        "i_idx": i_idx,
    }
    in_maps, s1, mse = _host_prep(inputs)
    nc = build_nc()
    res = run_bass_kernel_spmd(nc, in_maps, core_ids=list(range(NCORES)), trace=_trace)
    outs = np.stack([res.results[c]["out"] for c in range(NCORES)]).astype(np.float64)  # [8,128,3]
    S2 = outs[..., 0].sum()
    S3 = outs[..., 1:3].sum() / NCORES  # redundant per-core copies
    contrastive = (S2 + S3 - 2.0 * s1) / (2.0 * N)
    loss = ALPHA * contrastive + (1.0 - ALPHA) * mse
    if _trace:
        kernel._last_results = res
    return np.float32(loss)


# revision 29
# speedup vs baseline: 1.8437x; 1.0096x over previous
"""Trainium2 Bass kernel for nn_ContrastiveMSELoss (8192x8192 cos-sim contrastive + MSE).

Sharding: 8 NeuronCores, users row-sharded 1024/core, full (normalized,
transposed) recipe table per core.

The loss decomposes so the 8192x8192 ratings matrix is never materialized:
    rowR[i]  = 0.1*M + sum_{final scatter cells in row i}(v - 0.1)
    S1       = 0.1*T + sum_pairs (v-0.1)*cos[u,i],  T = (sum_i u_i/|u_i|) . (sum_j r_j/|r_j|)
    S2       = sum_i rowR[i] * log(rowsum_exp[i])
    S3       = sum_i rowR[i] * log(colsum_exp[i])    (col_sum indexed by i: torch n==m quirk)
    loss     = 0.5*(S2 + S3 - 2*S1)/(2*N) + 0.5*mean((ratings-cos_sim)^2)

v9: host prep in numpy (normalize+transpose embeddings, dedup scatter, and the
O(N*D)/O(B*D) scalar terms PAIR/T/MSE in f64); the device keeps the O(N^2) work.
Main loop, 4 column groups x 8 row tiles: K=64 bf16 matmuls into a double-
buffered [128,2048] f32 PSUM ring, one wide Exp ACTIVATE per tile (ACT is the
throughput floor: 8.4M exps/core at 1 elem/cycle/lane @1.2GHz) with row sums on
the ACT accumulator, column sums accumulated on DVE in bf16. Each group's
partition-reduce (col-tiled ones-matmuls into a pg-ring slot + bounce + DMA) is
deferred to the middle of the NEXT group, emitted before that iteration's pg so
it lands in an already-free ring slot. Cross-core column reduction: warmup
AllGather at t~0 (absorbs the CC cold barrier), stage-A AllGather mid-loop
(hidden), stage-B exposed at the tail with the last ex tile folded into the
reduce on PE, the PSUM bounce split across ACT+DVE, cc_in DMAs split across both
HWDGE queues, and every core redundantly reducing all 8 partials (no
core-dependent addressing). S2/S3 exit as per-partition partial vectors, each
DMA'd the moment its reduce lands (S2 and stage-A during the AllGather wait);
the host does the final partition/core sums in f64.
"""

import sys

sys.path.insert(0, "/opt/trn_rl_repo")

import numpy as np

import concourse.bass as bass
import concourse.bacc as bacc
import concourse.tile as tile
from concourse import mybir
from concourse.bass_utils import run_bass_kernel_spmd

f32 = mybir.dt.float32
bf16 = mybir.dt.bfloat16
AF = mybir.ActivationFunctionType
OP = mybir.AluOpType
AX = mybir.AxisListType

NCORES = 8
N = 8192          # users
M = 8192          # recipes
D = 64
B = 65536
S = N // NCORES   # slab rows per core (1024)
RT = S // 128     # row tiles per slab (8)
NG = 4            # column groups
GW = 2048         # group width (cols)
ALPHA = 0.5
FILL = 0.1

CS_P = (0, 32, 64, 0)    # ones-matmul output partition per 512-chunk
CS_F = (0, 0, 0, 512)    # ... and free offset (base_partition 96 is rejected)


def build_nc():
    nc = bacc.Bacc(num_devices=NCORES)

    ut_d = nc.declare_dram_parameter("ut", [D, S], bf16, isOutput=False)
    rt_d = nc.declare_dram_parameter("rt", [D, M], bf16, isOutput=False)
    rowr_slab_d = nc.declare_dram_parameter("rowr_slab", [S], f32, isOutput=False)
    rowr_full_d = nc.declare_dram_parameter("rowr_full", [M], f32, isOutput=False)
    out_d = nc.declare_dram_parameter("out", [3, 128], f32, isOutput=True)

    with tile.TileContext(nc) as tc:
        with tc.tile_pool(name="sb", bufs=1) as sb, \
             tc.tile_pool(name="dram", bufs=1, space="DRAM") as dpool:
            cc_in = dpool.tile([M], f32)
            cc_allA = dpool.tile([NCORES, 4096], f32)
            cc_allB = dpool.tile([NCORES, 4096], f32)
            warm_in = dpool.tile([16], f32)
            warm_out = dpool.tile([16 * NCORES], f32)

            # ---- highest-priority loads: exactly what the first matmuls
            # need first, split across both HWDGE queues ----
            ut_sb = sb.tile([D, S], bf16)
            rt_sb = sb.tile([D, M], bf16)
            nc.sync.dma_start(out=ut_sb[:, 0:128], in_=ut_d[:, 0:128])
            nc.scalar.dma_start(out=rt_sb[:, 1024:2048], in_=rt_d[:, 1024:2048])
            nc.sync.dma_start(out=rt_sb[:, 0:512], in_=rt_d[:, 0:512])
            nc.sync.dma_start(out=rt_sb[:, 512:1024], in_=rt_d[:, 512:1024])
            nc.sync.dma_start(out=ut_sb[:, 128:S], in_=ut_d[:, 128:S])

            ones_bf = sb.tile([128, 1], bf16)
            nc.vector.memset(ones_bf[:], 1.0)

            # prime the CC channel early; nothing depends on the result
            wz = sb.tile([1, 16], f32)
            nc.vector.memset(wz[:], 0.0)
            nc.sync.dma_start(out=warm_in[:], in_=wz[:])
            nc.gpsimd.collective_compute(
                "AllGather", OP.bypass,
                replica_groups=[list(range(NCORES))],
                ins=[warm_in[:].opt()], outs=[warm_out[:].opt()])

            for g in range(1, NG):
                nc.sync.dma_start(
                    out=rt_sb[:, g * GW:(g + 1) * GW], in_=rt_d[:, g * GW:(g + 1) * GW])
            rowr2 = sb.tile([128, RT], f32)
            nc.sync.dma_start(out=rowr2[:], in_=rowr_slab_d[:].rearrange("(r p) -> p r", p=128))
            # rowR keyed by colsum index, per AllGather stage (A: cols 0-4096,
            # B: 4096-8192); within a stage col j = off + p*32 + b
            rowrA = sb.tile([128, 32], f32)
            nc.sync.dma_start(out=rowrA[:], in_=rowr_full_d[0:4096].rearrange("(p b) -> p b", p=128))
            rowrB = sb.tile([128, 32], f32)
            nc.sync.dma_start(out=rowrB[:], in_=rowr_full_d[4096:8192].rearrange("(p b) -> p b", p=128))

            rs_parts = sb.tile([128, NG * RT], f32)
            acc = sb.tile([128, NG, GW], bf16)
            s3_acc = sb.tile([128, 2], f32)

            with tc.tile_pool(name="psM", bufs=1, space="PSUM") as psM:

                def colsum_reduce(g, tail=False, extra=None):
                    """Partition-reduce acc[:, g] (128 -> 1) via 4 col-tiled
                    ones-matmuls into a pg-ring PSUM slot, bounce, DMA to cc_in.
                    `extra` is an unaccumulated ex tile folded in via a second
                    accumulating matmul (keeps the last DVE add off the tail)."""
                    csP = psM.tile([128, GW], f32, tag="cos", bufs=2, name=f"cs{g}")
                    for c in range(4):
                        nc.tensor.matmul(
                            out=csP[CS_P[c]:CS_P[c] + 1, CS_F[c]:CS_F[c] + 512],
                            lhsT=ones_bf[:, 0:1],
                            rhs=acc[:, g, c * 512:(c + 1) * 512],
                            start=True, stop=extra is None, skip_group_check=True)
                        if extra is not None:
                            nc.tensor.matmul(
                                out=csP[CS_P[c]:CS_P[c] + 1, CS_F[c]:CS_F[c] + 512],
                                lhsT=ones_bf[:, 0:1],
                                rhs=extra[:, c * 512:(c + 1) * 512],
                                start=False, stop=True, skip_group_check=True)
                    # split the PSUM->SBUF bounce across ACT+DVE: mid-loop ACT
                    # is stalled on the ring handoff here anyway (Copy is in
                    # the Exp table set, no reload), and the earlier bounce
                    # completion releases the ring slot sooner
                    bounce = sb.tile([128, 1024], f32, tag="csb", bufs=2, name=f"csb{g}")
                    nc.scalar.activation(
                        out=bounce[:, 0:512], in_=csP[:, 0:512], func=AF.Copy)
                    nc.vector.tensor_copy(out=bounce[:, 512:1024], in_=csP[:, 512:1024])
                    for c in range(4):
                        eng = nc.scalar if (tail and c % 2) else nc.sync
                        eng.dma_start(
                            out=cc_in[g * GW + c * 512: g * GW + (c + 1) * 512],
                            in_=bounce[CS_P[c]:CS_P[c] + 1, CS_F[c]:CS_F[c] + 512])

                for g in range(NG):
                    for r in range(RT):
                        # deferred reduce of the PREVIOUS group, emitted before
                        # this iteration's pg so csP lands in the ring slot
                        # whose ACT reader finished two iterations ago
                        if r == 4 and g >= 1:
                            colsum_reduce(g - 1)
                            if g == 2:
                                nc.gpsimd.collective_compute(
                                    "AllGather", OP.bypass,
                                    replica_groups=[list(range(NCORES))],
                                    ins=[cc_in[0:4096].opt()], outs=[cc_allA[:].opt()])
                        pg = psM.tile([128, GW], f32, tag="cos", bufs=2)
                        for c in range(4):
                            nc.tensor.matmul(
                                out=pg[:, c * 512:(c + 1) * 512],
                                lhsT=ut_sb[:, r * 128:(r + 1) * 128],
                                rhs=rt_sb[:, g * GW + c * 512: g * GW + (c + 1) * 512],
                                start=True, stop=True)
                        ex = sb.tile([128, GW], bf16, tag="exp", bufs=4)
                        idx = g * RT + r
                        nc.scalar.activation(
                            out=ex[:], in_=pg[:], func=AF.Exp,
                            accum_out=rs_parts[:, idx:idx + 1])
                        if r == 0:
                            nc.vector.tensor_copy(out=acc[:, g, :], in_=ex[:])
                        elif g == NG - 1 and r == RT - 1:
                            last_ex = ex  # folded into colsum_reduce(3) on PE
                        else:
                            nc.vector.tensor_tensor(
                                out=acc[:, g, :], in0=acc[:, g, :], in1=ex[:], op=OP.add)

                colsum_reduce(3, tail=True, extra=last_ex)
                nc.gpsimd.collective_compute(
                    "AllGather", OP.bypass,
                    replica_groups=[list(range(NCORES))],
                    ins=[cc_in[4096:8192].opt()], outs=[cc_allB[:].opt()])

            # =============== tail ===============
            if True:
                # stage-A gather finished mid-loop; its local reduce + ln +
                # dot run here, under the AG-B latency. (Kept out of the main
                # loop: a slow collective there blocks the DVE queue and
                # backs up into ACT.)
                tmpA = sb.tile([128, NCORES, 32], f32)
                nc.sync.dma_start(
                    out=tmpA[:], in_=cc_allA[:].rearrange("c (p b) -> p c b", p=128))
                colsA = sb.tile([128, 32], f32)
                nc.vector.tensor_reduce(
                    out=colsA[:], in_=tmpA[:].rearrange("p c b -> p b c"),
                    axis=AX.X, op=OP.add)

                # S2 path: runs on ACT/DVE while AllGather C is in flight
                rs_r = sb.tile([128, RT], f32)
                nc.vector.tensor_reduce(
                    out=rs_r[:], in_=rs_parts[:].rearrange("p (g r) -> p r g", g=NG),
                    axis=AX.X, op=OP.add)
                lrs = sb.tile([128, RT], f32)
                nc.scalar.activation(out=lrs[:], in_=rs_r[:], func=AF.Ln)
                s2w = sb.tile([128, RT], f32)
                nc.vector.tensor_tensor(out=s2w[:], in0=lrs[:], in1=rowr2[:], op=OP.mult)
                s2_acc = sb.tile([128, 1], f32)
                nc.vector.tensor_reduce(out=s2_acc[:], in_=s2w[:], axis=AX.X, op=OP.add)
                nc.sync.dma_start(out=out_d[0, :], in_=s2_acc[:])

                lnA = sb.tile([128, 32], f32)
                nc.scalar.activation(out=lnA[:], in_=colsA[:], func=AF.Ln)
                w3A = sb.tile([128, 32], f32)
                nc.vector.tensor_tensor(out=w3A[:], in0=lnA[:], in1=rowrA[:], op=OP.mult)
                nc.vector.tensor_reduce(out=s3_acc[:, 0:1], in_=w3A[:], axis=AX.X, op=OP.add)
                nc.sync.dma_start(out=out_d[1, :], in_=s3_acc[:, 0:1])
                # stage B: gather + reduce + ln + dot (exposed)
                tmpB = sb.tile([128, NCORES, 32], f32)
                nc.sync.dma_start(
                    out=tmpB[:], in_=cc_allB[:].rearrange("c (p b) -> p c b", p=128))
                colsB = sb.tile([128, 32], f32)
                nc.vector.tensor_reduce(
                    out=colsB[:], in_=tmpB[:].rearrange("p c b -> p b c"),
                    axis=AX.X, op=OP.add)
                lnB = sb.tile([128, 32], f32)
                nc.scalar.activation(out=lnB[:], in_=colsB[:], func=AF.Ln)
                w3B = sb.tile([128, 32], f32)
                nc.vector.tensor_tensor(out=w3B[:], in0=lnB[:], in1=rowrB[:], op=OP.mult)
                nc.vector.tensor_reduce(out=s3_acc[:, 1:2], in_=w3B[:], axis=AX.X, op=OP.add)
                nc.sync.dma_start(out=out_d[2, :], in_=s3_acc[:, 1:2])
    nc.finalize()
    return nc


def _host_prep(inputs):
    """Normalize+transpose embeddings, dedup scatter (last write wins), and the
    O(N*D)/O(B*D) scalar terms (PAIR, T, MSE) in f64 numpy."""
    U = np.asarray(inputs["user_embeddings"], dtype=np.float32)
    R = np.asarray(inputs["recipe_embeddings"], dtype=np.float32)
    rat = np.asarray(inputs["ratings_scaled"], dtype=np.float32)
    css = np.asarray(inputs["cos_similarities_scaled"], dtype=np.float32)
    u = np.asarray(inputs["u_idx"]).astype(np.int64)
    i = np.asarray(inputs["i_idx"]).astype(np.int64)

    Uh = U.astype(np.float64)
    Rh = R.astype(np.float64)
    un = np.maximum(np.linalg.norm(Uh, axis=1), 1e-8)
    rn = np.maximum(np.linalg.norm(Rh, axis=1), 1e-8)
    Uh /= un[:, None]
    Rh /= rn[:, None]

    import ml_dtypes
    ut_all = np.ascontiguousarray(Uh.T).astype(ml_dtypes.bfloat16)   # [64, N]
    rt = np.ascontiguousarray(Rh.T).astype(ml_dtypes.bfloat16)       # [64, M]

    cell = u * M + i
    _, idx_rev = np.unique(cell[::-1], return_index=True)
    keep = (B - 1 - idx_rev)  # last occurrences
    uu = u[keep]
    ii = i[keep]
    ww = (rat[keep].astype(np.float64) - FILL)

    delta = np.bincount(uu, weights=ww, minlength=N)
    row_r32 = (FILL * M + delta).astype(np.float32)

    pair = float(np.einsum("ij,ij->", Uh[uu] * ww[:, None], Rh[ii]))
    T = float(Uh.sum(0) @ Rh.sum(0))
    s1 = FILL * T + pair
    mse = float(np.mean((rat.astype(np.float64) - css.astype(np.float64)) ** 2))

    in_maps = []
    for c in range(NCORES):
        in_maps.append({
            "ut": np.ascontiguousarray(ut_all[:, c * S:(c + 1) * S]),
            "rt": rt,
            "rowr_slab": np.ascontiguousarray(row_r32[c * S:(c + 1) * S]),
            "rowr_full": row_r32,
        })
    return in_maps, s1, mse


def kernel(user_embeddings, recipe_embeddings, ratings_scaled, cos_similarities_scaled,
           u_idx, i_idx, _trace=False):
    inputs = {
        "user_embeddings": user_embeddings,
        "recipe_embeddings": recipe_embeddings,
        "ratings_scaled": ratings_scaled,
        "cos_similarities_scaled": cos_similarities_scaled,
        "u_idx": u_idx,
        "i_idx": i_idx,
    }
    in_maps, s1, mse = _host_prep(inputs)
    nc = build_nc()
    res = run_bass_kernel_spmd(nc, in_maps, core_ids=list(range(NCORES)), trace=_trace)
    outs = np.stack([res.results[c]["out"] for c in range(NCORES)]).astype(np.float64)  # [8,3,128]
    S2 = outs[:, 0, :].sum()
    S3 = outs[:, 1:3, :].sum() / NCORES  # redundant per-core copies
    contrastive = (S2 + S3 - 2.0 * s1) / (2.0 * N)
    loss = ALPHA * contrastive + (1.0 - ALPHA) * mse
    if _trace:
        kernel._last_results = res
    return np.float32(loss)
